# revision 9
# baseline (speedup 1.0000x reference)
import sys

sys.path.insert(0, "/opt/trn_rl_repo")

import numpy as np
import ml_dtypes

import concourse.bass as bass
from concourse import bacc
import concourse.tile as tile
import concourse.mybir as mybir
from concourse.bass_utils import run_bass_kernel_spmd

# Problem shape (hardcoded per contract)
B, T, D, H, DK = 4, 2048, 1024, 16, 64
NCORES = 8
TQ = T // 2  # query tokens per core (each batch is split across 2 cores)

P = 128
NDT = D // P      # 8 din/dout tiles
NKT = T // P      # 16 key-token tiles
NPAIR = H // 2    # 8 head pairs (pair p <-> dout tile p)
QCH = 512         # free-dim chunk per matmul
NQC = TQ // QCH   # 2 q-chunks per core
NTT = TQ // P     # 8 output token tiles

bf16 = mybir.dt.bfloat16
f32 = mybir.dt.float32
FT = mybir.ActivationFunctionType
ADD = mybir.AluOpType.add
MUL = mybir.AluOpType.mult

_CACHE = {}


def build_kernel():
    nc = bacc.Bacc("TRN2", target_bir_lowering=False, debug=False, num_devices=1)

    # Per-core inputs. Token order per core: [own query half | other half].
    # Attention sums over keys, so key/value token order is irrelevant.
    xTq = nc.dram_tensor("xTq", [D, TQ], bf16, kind="ExternalInput")  # own half, transposed
    xTo = nc.dram_tensor("xTo", [D, TQ], bf16, kind="ExternalInput")  # other half, transposed
    Wq = nc.dram_tensor("Wq", [D, D], bf16, kind="ExternalInput")
    Wk = nc.dram_tensor("Wk", [D, D], bf16, kind="ExternalInput")
    Wv = nc.dram_tensor("Wv", [D, D], bf16, kind="ExternalInput")
    Wo = nc.dram_tensor("Wo", [D, D], bf16, kind="ExternalInput")
    # bq/bk pre-striped on host to [128, NDT] (col t = bias[t*128:(t+1)*128])
    bqp = nc.dram_tensor("bqp", [P, NDT], f32, kind="ExternalInput")
    bkp = nc.dram_tensor("bkp", [P, NDT], f32, kind="ExternalInput")
    bv = nc.dram_tensor("bv", [1, D], f32, kind="ExternalInput")
    bo = nc.dram_tensor("bo", [1, D], f32, kind="ExternalInput")
    out = nc.dram_tensor("out", [TQ, D], f32, kind="ExternalOutput")

    with tile.TileContext(nc) as tc:
        with (
            tc.tile_pool(name="big", bufs=1) as big,
            tc.tile_pool(name="wst", bufs=3) as wst,
            tc.tile_pool(name="tmp", bufs=3) as tmp,
            tc.tile_pool(name="dram", bufs=1, space="DRAM") as dramp,
            tc.tile_pool(name="acc", bufs=2, space="PSUM") as accp,
            tc.tile_pool(name="sg", bufs=2, space="PSUM") as sgp,
            tc.tile_pool(name="ops", bufs=1, space="PSUM") as opsp,
        ):
            # ---------- persistent small tensors ----------
            bq_sb = big.tile([P, NDT], f32, name="bq_sb")
            bk_sb = big.tile([P, NDT], f32, name="bk_sb")
            nc.sync.dma_start(bq_sb[:], bqp[:])
            nc.sync.dma_start(bk_sb[:], bkp[:])
            bv_rep = big.tile([P, D], f32, name="bv_rep")
            bo_rep = big.tile([P, D], f32, name="bo_rep")
            nc.sync.dma_start(bv_rep[:], bv[:].to_broadcast((P, D)))
            nc.sync.dma_start(bo_rep[:], bo[:].to_broadcast((P, D)))

            # x^T tiles [128, T] per din tile; cols [0:TQ]=own half, [TQ:T]=other
            xt_sb = [big.tile([P, T], bf16, name=f"xt{i}") for i in range(NDT)]
            for i in range(NDT):
                nc.sync.dma_start(xt_sb[i][:, 0:TQ], xTq[i * P : (i + 1) * P, :])
                nc.sync.dma_start(xt_sb[i][:, TQ:T], xTo[i * P : (i + 1) * P, :])

            # persistent activations
            kt_sb = [big.tile([P, T], bf16, name=f"kt{p}") for p in range(NPAIR)]
            qt_sb = [big.tile([P, TQ], bf16, name=f"qt{p}") for p in range(NPAIR)]
            # V' padded per token tile: per head 65 cols.
            #   even head: [V(64) | 1] ; odd head: [1 | V(64)]
            vp_sb = [big.tile([P, H, DK + 1], bf16, name=f"vp{t}") for t in range(NKT)]
            for t in range(NKT):
                nc.any.memset(vp_sb[t][:], 1.0)
            # attention output O^T (bf16), raw then normalized in place
            ob_sb = [
                [big.tile([P, QCH], bf16, name=f"ob{p}_{c}") for c in range(NQC)]
                for p in range(NPAIR)
            ]
            # softmax denominators / reciprocals per q-chunk: row h = head h
            den_sb = [big.tile([H, QCH], f32, name=f"den{c}") for c in range(NQC)]
            rec_sb = [big.tile([H, QCH], f32, name=f"rec{c}") for c in range(NQC)]
            rec_dr = [dramp.tile([H, QCH], f32, name=f"recd{c}") for c in range(NQC)]
            # weight chunk tiles (reused across the two dout chunks)
            wv_ch = big.tile([P, NDT, QCH], bf16, name="wv_ch")
            wo_ch = big.tile([P, NDT, QCH], bf16, name="wo_ch")

            # ---------- phase 1: projections ----------
            def proj_T(w_dram, bias_sb, dst_tiles, rhs_ap, ntok):
                # dst[dt][128, ntok] = (W[:, dt*128:+128]^T @ x^T) + bias
                nch = ntok // QCH
                for dt in range(NDT):
                    w_t = wst.tile([P, NDT, P], bf16, tag="wstream")
                    nc.sync.dma_start(
                        w_t[:],
                        w_dram[:, dt * P : (dt + 1) * P].rearrange(
                            "(a p) m -> p a m", p=P
                        ),
                    )
                    for ch in range(nch):
                        ps = accp.tile([P, QCH], f32, name="proj_ps")
                        for di in range(NDT):
                            nc.tensor.matmul(
                                ps[:],
                                w_t[:, di, :],
                                rhs_ap(di)[:, ch * QCH : (ch + 1) * QCH],
                                start=(di == 0),
                                stop=(di == NDT - 1),
                            )
                        nc.vector.tensor_tensor(
                            dst_tiles[dt][:, ch * QCH : (ch + 1) * QCH],
                            ps[:],
                            bias_sb[:, dt : dt + 1].to_broadcast((P, QCH)),
                            ADD,
                        )

            proj_T(Wk, bk_sb, kt_sb, lambda di: xt_sb[di], T)
            proj_T(Wq, bq_sb, qt_sb, lambda di: xt_sb[di][:, 0:TQ], TQ)

            # V in natural layout, scattered into the padded V' tiles
            for ch in range(2):  # dout chunks of 512 = 8 heads each
                nc.sync.dma_start(
                    wv_ch[:],
                    Wv[:, ch * QCH : (ch + 1) * QCH].rearrange("(a p) m -> p a m", p=P),
                )
                h0 = ch * 8
                for tt in range(NKT):
                    ps = accp.tile([P, QCH], f32, name="proj_ps")
                    for di in range(NDT):
                        nc.tensor.matmul(
                            ps[:],
                            xt_sb[di][:, tt * P : (tt + 1) * P],
                            wv_ch[:, di, :],
                            start=(di == 0),
                            stop=(di == NDT - 1),
                        )
                    # all heads laid out as [V(64) | 1]
                    nc.vector.tensor_tensor(
                        vp_sb[tt][:, h0 : h0 + 8, 0:DK],
                        ps[:].rearrange("p (h d) -> p h d", d=DK),
                        bv_rep[:, ch * QCH : (ch + 1) * QCH].rearrange(
                            "p (h d) -> p h d", d=DK
                        ),
                        ADD,
                    )

            # ---------- phase 2: attention ----------
            for c in range(NQC):
                qsl = slice(c * QCH, (c + 1) * QCH)
                for p in range(NPAIR):
                    hA, hB = 2 * p, 2 * p + 1
                    oA = opsp.tile([P, QCH], f32, name="oA")
                    oB = opsp.tile([P, QCH], f32, name="oB")
                    for g in range(NKT // 2):
                        sgA = sgp.tile([P, 2, QCH], f32, tag="sg")
                        sgB = sgp.tile([P, 2, QCH], f32, tag="sg")
                        for j in range(2):
                            kt = 2 * g + j
                            ksl = slice(kt * P, (kt + 1) * P)
                            nc.tensor.matmul(
                                sgA[:, j, :],
                                kt_sb[p][0:DK, ksl],
                                qt_sb[p][0:DK, qsl],
                                start=True,
                                stop=True,
                                tile_position=(0, 0),
                            )
                            nc.tensor.matmul(
                                sgB[:, j, :],
                                kt_sb[p][DK:P, ksl],
                                qt_sb[p][DK:P, qsl],
                                start=True,
                                stop=True,
                                tile_position=(64, 0),
                            )
                        ptA = tmp.tile([P, 2, QCH], bf16, tag="pt")
                        ptB = tmp.tile([P, 2, QCH], bf16, tag="pt")
                        nc.scalar.activation(ptA[:], sgA[:], FT.Exp, scale=0.125)
                        nc.scalar.activation(ptB[:], sgB[:], FT.Exp, scale=0.125)
                        for j in range(2):
                            kt = 2 * g + j
                            nc.tensor.matmul(
                                oA[0:65, :],
                                vp_sb[kt][:, hA, :],
                                ptA[:, j, :],
                                start=(kt == 0),
                                stop=(kt == NKT - 1),
                            )
                            nc.tensor.matmul(
                                oB[0:65, :],
                                vp_sb[kt][:, hB, :],
                                ptB[:, j, :],
                                start=(kt == 0),
                                stop=(kt == NKT - 1),
                            )
                    # raw O^T to SBUF (bf16), freeing the psum banks.
                    # Head B goes via a staging tile + partition-shift DMA
                    # (only needed by phase 3, so off the critical path).
                    # Denominators (row 64, from the "ones" column of V')
                    # stage through fp32 row tiles, then DMA to den_sb.
                    nc.vector.tensor_copy(ob_sb[p][c][0:DK, :], oA[0:DK, :])
                    stgB = tmp.tile([DK, QCH], bf16, tag="bstg")
                    nc.vector.tensor_copy(stgB[:], oB[0:DK, :])
                    nc.sync.dma_start(ob_sb[p][c][DK:P, :], stgB[:])
                    stgDA = tmp.tile([65, QCH], f32, tag="dstgA")
                    stgDB = tmp.tile([65, QCH], f32, tag="dstgB")
                    nc.vector.tensor_copy(stgDA[64:65, :], oA[64:65, :])
                    nc.vector.tensor_copy(stgDB[64:65, :], oB[64:65, :])
                    nc.sync.dma_start(den_sb[c][hA : hA + 1, :], stgDA[64:65, :])
                    nc.sync.dma_start(den_sb[c][hB : hB + 1, :], stgDB[64:65, :])

                # normalize: reciprocal of denominators, broadcast, multiply
                nc.vector.reciprocal(rec_sb[c][:], den_sb[c][:])
                nc.sync.dma_start(rec_dr[c][:], rec_sb[c][:])
                for p in range(NPAIR):
                    hA, hB = 2 * p, 2 * p + 1
                    rep = tmp.tile([P, QCH], f32, tag="rep")
                    nc.sync.dma_start(
                        rep[0:DK, :], rec_dr[c][hA : hA + 1, :].to_broadcast((DK, QCH))
                    )
                    nc.sync.dma_start(
                        rep[DK:P, :], rec_dr[c][hB : hB + 1, :].to_broadcast((DK, QCH))
                    )
                    nc.vector.tensor_tensor(
                        ob_sb[p][c][:], ob_sb[p][c][:], rep[:], MUL
                    )

            # ---------- phase 3: output projection ----------
            for ch in range(2):
                nc.sync.dma_start(
                    wo_ch[:],
                    Wo[:, ch * QCH : (ch + 1) * QCH].rearrange("(a p) m -> p a m", p=P),
                )
                for ttk in range(NTT):
                    c, s = ttk // 4, (ttk % 4) * P
                    ps = accp.tile([P, QCH], f32, name="proj_ps")
                    for p in range(NPAIR):
                        nc.tensor.matmul(
                            ps[:],
                            ob_sb[p][c][:, s : s + P],
                            wo_ch[:, p, :],
                            start=(p == 0),
                            stop=(p == NPAIR - 1),
                        )
                    res = tmp.tile([P, QCH], f32, tag="ores")
                    nc.vector.tensor_tensor(
                        res[:], ps[:], bo_rep[:, ch * QCH : (ch + 1) * QCH], ADD
                    )
                    nc.sync.dma_start(
                        out[ttk * P : (ttk + 1) * P, ch * QCH : (ch + 1) * QCH],
                        res[:],
                    )

    nc.compile()
    return nc


def _prep_inputs(x, Wq, bq, Wk, bk, Wv, bv, Wo, bo):
    """Shard + lay out inputs for the 8 cores."""
    x = np.asarray(x, dtype=np.float32)
    to_bf = lambda a: np.ascontiguousarray(a).astype(ml_dtypes.bfloat16)
    Wq_b, Wk_b, Wv_b, Wo_b = (
        to_bf(np.asarray(w, np.float32)) for w in (Wq, Wk, Wv, Wo)
    )
    bqp = np.ascontiguousarray(np.asarray(bq, np.float32).reshape(NDT, P).T)
    bkp = np.ascontiguousarray(np.asarray(bk, np.float32).reshape(NDT, P).T)
    bv_r = np.ascontiguousarray(np.asarray(bv, np.float32).reshape(1, D))
    bo_r = np.ascontiguousarray(np.asarray(bo, np.float32).reshape(1, D))
    in_maps = []
    for core in range(NCORES):
        b, half = core // 2, core % 2
        xTb = to_bf(x[b].T)  # [D, T]
        own = np.ascontiguousarray(xTb[:, half * TQ : (half + 1) * TQ])
        other = np.ascontiguousarray(xTb[:, (1 - half) * TQ : (2 - half) * TQ])
        in_maps.append(
            {
                "xTq": own,
                "xTo": other,
                "Wq": Wq_b,
                "Wk": Wk_b,
                "Wv": Wv_b,
                "Wo": Wo_b,
                "bqp": bqp,
                "bkp": bkp,
                "bv": bv_r,
                "bo": bo_r,
            }
        )
    return in_maps


def kernel(x, Wq, bq, Wk, bk, Wv, bv, Wo, bo):
    if "nc" not in _CACHE:
        _CACHE["nc"] = build_kernel()
    nc = _CACHE["nc"]
    in_maps = _prep_inputs(x, Wq, bq, Wk, bk, Wv, bv, Wo, bo)
    res = run_bass_kernel_spmd(nc, in_maps, list(range(NCORES)))
    out = np.empty((B, T, D), dtype=np.float32)
    for core in range(NCORES):
        b, half = core // 2, core % 2
        out[b, half * TQ : (half + 1) * TQ, :] = res.results[core]["out"]
    return out
